# revision 21
# baseline (speedup 1.0000x reference)
"""nn_AttentionPool Trainium2 kernel.

kernel(x, batch, W1, b1, W2, b2) -> np.ndarray [2048, 1024] float32

Strategy (8 NeuronCores, SPMD, data-parallel over node rows; batch is
sorted so each core covers a contiguous segment range):
  - Host ships per core: x in fp16 twice, pre-tiled for contiguous DMA —
    node-major (scatter operand) and dim-major (MLP moving operand).
  - Per 4-tile group (512 nodes) on device (3-stage pipeline, staggered
    2 groups per stage so PE never waits on ACT/DVE latency):
      stage A (mlp):   PE  hT[hh] += W1[:,c,hh]^T @ xT[:,c]  (fp16, N=512)
                       ACT thT = tanh(hT + b1) -> fp16
      stage B (score): DVE mm = th0*w2c0 + th1*w2c1          (fp16)
                       PE  s[i]  = mm_slice^T @ ones          (colsum = per-
                            node score, transposed onto partitions)
                       DVE sb = s - 25
                       DVE mg[i] = (iota==rel)*(b2+25)        (fp16 one-hot)
                       ACT A[i] = exp(mg + sb), accum_out -> e[:,t]
                            (match -> exp(s+b2); miss -> exp(s-25) ~ 0)
      stage C (scatter): PE u_win += A^T @ x  (fp16, 2x N=512, static
                            node-window PSUM accumulators; 3 windows/core)
  - Host: accumulates window outputs by true segment base, builds
    denominators from e, divides (reference epsilon semantics).
Max-shift note: s in [-1.2, 1.2] for this model so unshifted exp is safe;
softmax normalization cancels any constant shift.
"""
import os
import sys
import types

import ml_dtypes
import numpy as np

P = 128
DIM = 1024
HID = 256
NCH = DIM // P
NHH = HID // P
GRP = 4
N_CORES = 8
NUM_SEG = 2048

# ---------------------------------------------------------------------------
# environment compat (axon-tunneled trn2 + this walrus build)
# ---------------------------------------------------------------------------

def _install_ntff_hook():
    """antenv.axon_hooks is absent in this image; reconstruct it so
    trace=True (KERNEL_TRACE=1) can profile. Harmless if unused."""
    if "antenv.axon_hooks" in sys.modules:
        return
    m = types.ModuleType("antenv.axon_hooks")
    m._hook = None
    m.set_axon_ntff_profile_hook = lambda h: setattr(m, "_hook", h)
    m.get_axon_ntff_profile_hook = lambda: m._hook
    sys.modules["antenv.axon_hooks"] = m
    try:
        from trn_agent_boot.trn_boot import _ntff_profile_via_ctypes
        m.set_axon_ntff_profile_hook(
            _ntff_profile_via_ctypes("/opt/axon/libaxon_pjrt.so"))
    except Exception:
        pass


def _install_tile_compat():
    """This walrus accepts at most ONE sem wait per instruction; Tile's exit
    drain carries one per live proc. Patch the drain to spread waits."""
    from concourse import mybir
    from concourse.tile import TileContext, ScopedClock

    if getattr(TileContext, "_attnpool_patched", False):
        return

    def _patched(self, tick_clock, wait_clock):
        drain_inst = self.nc.sync.drain()
        wait_clock.add_sem_waits(
            drain_inst.ins, ScopedClock({None: tick_clock.global_clock}))
        si = drain_inst.ins.sync_info
        waits = list(si.on_wait or [])
        if len(waits) > 1:
            si.on_wait = waits[:1]
            for i, w in enumerate(waits[1:]):
                nop = self.nc.sync.nop(nofuse=True, hint=f"tailwait{i}")
                nop.ins.sync_info = mybir.SyncInfo(on_wait=[w], on_update=[])
        self.nc.all_engine_barrier()
        popped = self.nc._tile_sem_poison_stack.pop()
        assert popped is self._sem_poison
        self.nc.clear_and_free_semaphores(list(self.sems.allocated().values()))
        self.nc.all_engine_barrier()

    TileContext._drain_and_barrier = _patched
    TileContext._attnpool_patched = True


def _split_multi_waits(nc):
    """Post-pass: hoist extra sem waits onto single-wait NOPs."""
    from concourse import mybir
    n = 0
    for f in nc.m.functions:
        for blk in f.blocks:
            new = []
            for inst in blk.instructions:
                si = inst.sync_info
                waits = list(si.on_wait or []) if si else []
                if len(waits) > 1:
                    for w in waits[:-1]:
                        n += 1
                        nop = mybir.InstNoOp(name=f"I-waitsplit{n}", ins=[], outs=[])
                        nop.engine = inst.engine
                        nop.sync_info = mybir.SyncInfo(on_wait=[w], on_update=[])
                        new.append(nop)
                    si.on_wait = waits[-1:]
                new.append(inst)
            blk.instructions = new


# ---------------------------------------------------------------------------
# device program
# ---------------------------------------------------------------------------

def _build_kernel(NT, windows, b2_plus_25):
    from concourse import bass, mybir
    import concourse.tile as tile
    from concourse import hw_specs

    # The scheduler's cost model under-prices ScalarE/VectorE ops ~2x vs
    # measured HW (the 2x 16-bit accel never applies to fp32-PSUM reads,
    # and per-op overhead is higher than modeled), so it schedules their
    # results just-in-time and the real PE stalls on them every group.
    # Inflate the modeled access cost to measured reality so the
    # scheduler builds in the right slack.
    _ac = dict(hw_specs.TRN2Spec.ACCESS_CYCLES)
    _ac[(bass.MemorySpace.SBUF, mybir.EngineType.Activation)] = 400
    _ac[(bass.MemorySpace.PSUM, mybir.EngineType.Activation)] = 330
    _ac[(bass.MemorySpace.SBUF, mybir.EngineType.DVE)] = 208
    _ac[(bass.MemorySpace.PSUM, mybir.EngineType.DVE)] = 250
    hw_specs.TRN2Spec.ACCESS_CYCLES = _ac

    f32 = mybir.dt.float32
    fp16 = mybir.dt.float16

    nc = bass.Bass()
    NW = len(windows)
    NG = NT // GRP

    fp8 = mybir.dt.float8e4
    x_in = nc.declare_dram_parameter("x", [NG, P, GRP * DIM], fp16, isOutput=False)
    xT_in = nc.declare_dram_parameter("xT", [NG, P, NCH * GRP * P], fp8, isOutput=False)
    rel_in = nc.declare_dram_parameter("rel", [P, NT], f32, isOutput=False)
    w1_in = nc.declare_dram_parameter("w1", [DIM, HID], fp8, isOutput=False)
    w2c_in = nc.declare_dram_parameter("w2c", [P, NHH], f32, isOutput=False)
    b1_in = nc.declare_dram_parameter("b1", [HID], f32, isOutput=False)
    iota_in = nc.declare_dram_parameter("iota", [P, P], fp16, isOutput=False)
    ones_in = nc.declare_dram_parameter("ones", [P, 1], fp16, isOutput=False)
    neg25_in = nc.declare_dram_parameter("neg25", [P, 1], f32, isOutput=False)
    u_out = nc.declare_dram_parameter("u", [NW, P, DIM], f32, isOutput=True)
    s_out = nc.declare_dram_parameter("s", [P, NT], f32, isOutput=True)

    win_start = {a: w for w, (a, b) in enumerate(windows)}
    win_end = {b - 1: w for w, (a, b) in enumerate(windows)}
    tile_win = {}
    for w, (a, b) in enumerate(windows):
        for t in range(a, b):
            tile_win[t] = w

    with tile.TileContext(nc) as tc:
        with tc.tile_pool(name="const", bufs=1) as const, \
             tc.tile_pool(name="xpool", bufs=8) as xpool, \
             tc.tile_pool(name="xtpool", bufs=4) as xtpool, \
             tc.tile_pool(name="thpool", bufs=4) as thpool, \
             tc.tile_pool(name="mpool", bufs=4) as mpool, \
             tc.tile_pool(name="apool", bufs=4) as apool, \
             tc.tile_pool(name="mgpool", bufs=4) as mgpool, \
             tc.tile_pool(name="spool", bufs=3) as spool, \
             tc.tile_pool(name="opool", bufs=2) as opool, \
             tc.tile_pool(name="pp_h", bufs=4, space="PSUM") as pp_h, \
             tc.tile_pool(name="pp_s", bufs=2, space="PSUM") as pp_s, \
             tc.tile_pool(name="pp_u", bufs=1, space="PSUM") as pp_u:

            w1t = const.tile([P, NCH, HID], fp8)
            nc.sync.dma_start(out=w1t[:], in_=w1_in.rearrange("(c p) h -> p c h", p=P))
            w2c = const.tile([P, NHH], f32)
            nc.sync.dma_start(out=w2c[:], in_=w2c_in[:])
            b1t = const.tile([P, NHH], f32)
            nc.sync.dma_start(out=b1t[:], in_=b1_in.rearrange("(c p) -> p c", p=P))
            iota = const.tile([P, P], fp16)
            nc.sync.dma_start(out=iota[:], in_=iota_in[:])
            ones = const.tile([P, 1], fp16)
            nc.sync.dma_start(out=ones[:], in_=ones_in[:])
            neg25 = const.tile([P, 1], f32)
            nc.sync.dma_start(out=neg25[:], in_=neg25_in[:])
            relt = const.tile([P, NT], f32)
            nc.sync.dma_start(out=relt[:], in_=rel_in[:])
            s_stage = const.tile([P, NT], f32)

            state = {}
            ugroups = {}

            def emit_mlp(g):
                xg = xpool.tile([P, GRP, DIM], fp16, tag="xg")
                nc.sync.dma_start(out=xg[:].rearrange("p t d -> p (t d)"), in_=x_in[g])
                xTg = xtpool.tile([P, NCH, GRP * P], fp8, tag="xTg")
                nc.sync.dma_start(out=xTg[:].rearrange("p c n -> p (c n)"), in_=xT_in[g])

                thT = thpool.tile([P, NHH, GRP * P], fp16, tag="thT")
                for hh in range(NHH):
                    hTp = pp_h.tile([P, GRP * P], f32, tag="hT", name=f"hTp{hh}")
                    for c in range(0, NCH, 2):
                        nc.tensor.matmul(
                            hTp[:],
                            lhsT=w1t[:, c:c + 2, hh * P:(hh + 1) * P],
                            rhs=xTg[:, c:c + 2],
                            start=(c == 0), stop=(c == NCH - 2),
                            perf_mode=mybir.MatmulPerfMode.DoubleRow)
                    nc.scalar.activation(
                        thT[:, hh], hTp[:],
                        mybir.ActivationFunctionType.Tanh,
                        bias=b1t[:, hh:hh + 1])
                state[g] = {"xg": xg, "thT": thT}

            def emit_score1(g):
                t0 = g * GRP
                st = state[g]
                ctx = tc.high_priority()
                ctx.__enter__()
                # one-hot masks (constants only -> never blocks the DVE queue)
                mg = mgpool.tile([P, GRP, P], fp16, tag="mg")
                for i in range(GRP):
                    nc.vector.tensor_scalar(
                        out=mg[:, i],
                        in0=iota[:],
                        scalar1=relt[:, t0 + i:t0 + i + 1],
                        scalar2=float(b2_plus_25),
                        op0=mybir.AluOpType.is_equal,
                        op1=mybir.AluOpType.mult)
                # mm[p, n] = w2[p]*th0[p, n] + w2[128+p]*th1[p, n]
                m0 = mpool.tile([P, GRP * P], fp16, tag="m0")
                nc.vector.tensor_scalar(
                    out=m0[:], in0=st["thT"][:, 0], scalar1=w2c[:, 0:1],
                    scalar2=None, op0=mybir.AluOpType.mult)
                mm = mpool.tile([P, GRP * P], fp16, tag="mm")
                nc.vector.scalar_tensor_tensor(
                    out=mm[:], in0=st["thT"][:, 1], scalar=w2c[:, 1:2],
                    in1=m0[:], op0=mybir.AluOpType.mult,
                    op1=mybir.AluOpType.add)
                st["mg"] = mg
                st["mm"] = mm
                ctx.__exit__(None, None, None)

            def emit_colsum(g):
                st = state[g]
                # s per node, transposed onto partitions via colsum matmul
                sp = pp_s.tile([P, GRP], f32, tag="sp")
                for i in range(GRP):
                    nc.tensor.matmul(
                        sp[:, i:i + 1],
                        lhsT=st["mm"][:, i * P:(i + 1) * P],
                        rhs=ones[:],
                        start=True, stop=True)
                st["sp"] = sp

            def emit_score2(g):
                t0 = g * GRP
                st = state[g]
                ctx = tc.high_priority()
                ctx.__enter__()
                # s-25 lands in the persistent staging tile (shipped to host,
                # which rebuilds e = exp(s) and the denominators from it) and
                # doubles as the per-tile exp bias. On ScalarE: keeps the DVE
                # queue free of ops that wait on same-iteration PE results.
                nc.scalar.activation(
                    s_stage[:, t0:t0 + GRP], st["sp"][:],
                    mybir.ActivationFunctionType.Identity,
                    bias=neg25[:, 0:1])
                A = apool.tile([P, GRP, P], fp16, tag="A")
                for i in range(GRP):
                    nc.scalar.activation(
                        A[:, i], st["mg"][:, i],
                        mybir.ActivationFunctionType.Exp,
                        bias=s_stage[:, t0 + i:t0 + i + 1])
                st["A"] = A
                ctx.__exit__(None, None, None)

            def emit_scatter(g):
                ts = [g * GRP + i for i in range(GRP)]
                st = state[g]
                for i, t in enumerate(ts):
                    xt = st["xg"][:, i]
                    w = tile_win[t]
                    if t in win_start:
                        uwin = pp_u.tile([P, DIM], f32, tag="uwin")
                        ugroups[w] = uwin
                    up = ugroups[w]
                    for half in range(2):
                        nc.tensor.matmul(
                            up[:, half * 512:(half + 1) * 512],
                            lhsT=st["A"][:, i],
                            rhs=xt[:, half * 512:(half + 1) * 512],
                            start=(t in win_start), stop=(t in win_end))
                    if t in win_end:
                        us = opool.tile([P, DIM], f32, tag="us")
                        nc.vector.tensor_copy(us[:, 0:512], up[:, 0:512])
                        nc.scalar.copy(us[:, 512:1024], up[:, 512:1024])
                        nc.sync.dma_start(out=u_out[w], in_=us[:])
                del state[g]

            # per iteration g: every PE input is >=1 full iteration old.
            #   Scalar: sb(g-3)+exp(g-3) first, then tanh(g)
            #   PE:  mlp(g), colsum(g-2), scatter(g-5)
            #   DVE: mg/m0/mm(g-1), e-reduce(g-5)
            for g in range(NG + 5):
                if 3 <= g < NG + 3:
                    emit_score2(g - 3)
                if g < NG:
                    emit_mlp(g)
                if 1 <= g < NG + 1:
                    emit_score1(g - 1)
                if 2 <= g < NG + 2:
                    emit_colsum(g - 2)
                if 5 <= g:
                    emit_scatter(g - 5)

            nc.sync.dma_start(out=s_out[:], in_=s_stage[:])

    return nc


# ---------------------------------------------------------------------------
# host wrapper
# ---------------------------------------------------------------------------

def _make_windows(NT, nw):
    base, rem = divmod(NT, nw)
    sizes = [base + (1 if i < rem else 0) for i in range(nw)]
    out, a = [], 0
    for s in sizes:
        out.append((a, a + s))
        a += s
    return out


def _reference_numpy(x, batch, W1, b1, W2, b2):
    """Fallback for inputs outside this kernel's structural assumptions."""
    h = np.tanh(x.astype(np.float64) @ W1.astype(np.float64) + b1)
    s = (h @ W2.astype(np.float64) + b2).ravel()
    e = np.exp(s - s.max())
    denom = np.zeros(NUM_SEG, dtype=np.float64)
    np.add.at(denom, batch, e)
    attn = e / (denom[batch] + 1e-8)
    out = np.zeros((NUM_SEG, x.shape[1]), dtype=np.float64)
    np.add.at(out, batch, attn[:, None] * x.astype(np.float64))
    return out.astype(np.float32)


def kernel(x, batch, W1, b1, W2, b2):
    x = np.ascontiguousarray(np.asarray(x, dtype=np.float32))
    batch64 = np.asarray(batch).astype(np.int64)
    W1 = np.asarray(W1, dtype=np.float32)
    b1 = np.asarray(b1, dtype=np.float32)
    W2 = np.asarray(W2, dtype=np.float32)
    b2 = np.asarray(b2, dtype=np.float32)

    N = x.shape[0]
    pc = N // N_CORES
    NT = pc // P
    NG = NT // GRP if NT else 0

    ok = (N == N_CORES * pc and pc == NT * P and NT % GRP == 0
          and x.shape[1] == DIM and W1.shape == (DIM, HID)
          and np.all(batch64[:-1] <= batch64[1:])
          and batch64.min() >= 0 and batch64.max() < NUM_SEG)
    if not ok:
        return _reference_numpy(x, batch64, W1, b1, W2, b2)

    windows = _make_windows(NT, 3)
    iota = np.tile(np.arange(P, dtype=np.float16), (P, 1))
    b2f = float(b2.reshape(-1)[0])
    w2col = np.ascontiguousarray(
        W2.reshape(NHH, P).T.astype(np.float32))          # [P, NHH]
    ones16 = np.ones((P, 1), dtype=np.float16)

    in_maps, meta = [], []
    for c in range(N_CORES):
        bb = batch64[c * pc:(c + 1) * pc]
        rel = np.empty((NT, P), dtype=np.float32)
        bases = []
        for w, (a, b) in enumerate(windows):
            base = int(bb[a * P])
            bases.append(base)
            seg_rel = bb[a * P:b * P] - base
            if seg_rel.min() < 0 or seg_rel.max() >= P:
                return _reference_numpy(x, batch64, W1, b1, W2, b2)
            rel[a:b] = seg_rel.reshape(b - a, P).astype(np.float32)
        xc = x[c * pc:(c + 1) * pc]
        x16 = xc.astype(np.float16)
        x8 = xc.astype(ml_dtypes.float8_e4m3)
        xt = np.ascontiguousarray(
            x16.reshape(NG, GRP, P, DIM).transpose(0, 2, 1, 3)
               .reshape(NG, P, GRP * DIM))
        xTt = np.ascontiguousarray(
            x8.reshape(NG, GRP * P, NCH, P).transpose(0, 3, 2, 1)
               .reshape(NG, P, NCH * GRP * P))
        in_maps.append({
            "x": xt,
            "xT": xTt,
            "rel": np.ascontiguousarray(rel.T),
            "w1": W1.astype(ml_dtypes.float8_e4m3),
            "w2c": w2col,
            "b1": b1,
            "iota": iota,
            "ones": ones16,
            "neg25": np.full((P, 1), -25.0, dtype=np.float32),
        })
        meta.append(bases)

    _install_ntff_hook()
    _install_tile_compat()
    from concourse.bass_utils import run_bass_kernel_spmd

    nc = _build_kernel(NT, windows, b2f + 25.0)
    _split_multi_waits(nc)

    trace = os.environ.get("KERNEL_TRACE", "") == "1"
    res = run_bass_kernel_spmd(nc, in_maps, list(range(N_CORES)), trace=trace)
    if trace and res.exec_time_ns:
        print(f"[kernel] HW exec time: {res.exec_time_ns} ns", file=sys.stderr)
        kernel.last_exec_time_ns = res.exec_time_ns

    # host unshard: accumulate windows, build denominators from e, divide
    u_sum = np.zeros((NUM_SEG, DIM), dtype=np.float64)
    e_full = np.empty(N, dtype=np.float64)
    for c in range(N_CORES):
        r = res.results[c]
        s25 = r["s"].T.reshape(-1).astype(np.float64)
        e_full[c * pc:(c + 1) * pc] = np.exp(
            s25 + 25.0 + np.float64(np.float16(b2f + 25.0)) - 25.0)
        for w in range(len(windows)):
            base = meta[c][w]
            hi = min(base + P, NUM_SEG)
            u_sum[base:hi] += r["u"][w][:hi - base]
    denom = np.zeros(NUM_SEG, dtype=np.float64)
    np.add.at(denom, batch64, e_full)
    s_max = float(np.log(max(e_full.max(), 1e-30)))
    out = u_sum / (denom + 1e-8 * np.exp(s_max))[:, None]
    return out.astype(np.float32)


kernel.last_exec_time_ns = None


# revision 22
# speedup vs baseline: 1.0823x; 1.0823x over previous
"""nn_AttentionPool Trainium2 kernel.

kernel(x, batch, W1, b1, W2, b2) -> np.ndarray [2048, 1024] float32

Strategy (8 NeuronCores, SPMD, data-parallel over node rows; batch is
sorted so each core covers a contiguous segment range):
  - Host ships per core: x in fp16 twice, pre-tiled for contiguous DMA —
    node-major (scatter operand) and dim-major (MLP moving operand).
  - Per 4-tile group (512 nodes) on device (3-stage pipeline, staggered
    2 groups per stage so PE never waits on ACT/DVE latency):
      stage A (mlp):   PE  hT[hh] += W1[:,c,hh]^T @ xT[:,c]  (fp16, N=512)
                       ACT thT = tanh(hT + b1) -> fp16
      stage B (score): DVE mm = th0*w2c0 + th1*w2c1          (fp16)
                       PE  s[i]  = mm_slice^T @ ones          (colsum = per-
                            node score, transposed onto partitions)
                       DVE sb = s - 25
                       DVE mg[i] = (iota==rel)*(b2+25)        (fp16 one-hot)
                       ACT A[i] = exp(mg + sb), accum_out -> e[:,t]
                            (match -> exp(s+b2); miss -> exp(s-25) ~ 0)
      stage C (scatter): PE u_win += A^T @ x  (fp16, 2x N=512, static
                            node-window PSUM accumulators; 3 windows/core)
  - Host: accumulates window outputs by true segment base, builds
    denominators from e, divides (reference epsilon semantics).
Max-shift note: s in [-1.2, 1.2] for this model so unshifted exp is safe;
softmax normalization cancels any constant shift.
"""
import os
import sys
import types

import ml_dtypes
import numpy as np

P = 128
DIM = 1024
HID = 256
NCH = DIM // P
NHH = HID // P
GRP = 4
N_CORES = 8
NUM_SEG = 2048

# ---------------------------------------------------------------------------
# environment compat (axon-tunneled trn2 + this walrus build)
# ---------------------------------------------------------------------------

def _install_ntff_hook():
    """antenv.axon_hooks is absent in this image; reconstruct it so
    trace=True (KERNEL_TRACE=1) can profile. Harmless if unused."""
    if "antenv.axon_hooks" in sys.modules:
        return
    m = types.ModuleType("antenv.axon_hooks")
    m._hook = None
    m.set_axon_ntff_profile_hook = lambda h: setattr(m, "_hook", h)
    m.get_axon_ntff_profile_hook = lambda: m._hook
    sys.modules["antenv.axon_hooks"] = m
    try:
        from trn_agent_boot.trn_boot import _ntff_profile_via_ctypes
        m.set_axon_ntff_profile_hook(
            _ntff_profile_via_ctypes("/opt/axon/libaxon_pjrt.so"))
    except Exception:
        pass


def _install_tile_compat():
    """This walrus accepts at most ONE sem wait per instruction; Tile's exit
    drain carries one per live proc. Patch the drain to spread waits."""
    from concourse import mybir
    from concourse.tile import TileContext, ScopedClock

    if getattr(TileContext, "_attnpool_patched", False):
        return

    def _patched(self, tick_clock, wait_clock):
        drain_inst = self.nc.sync.drain()
        wait_clock.add_sem_waits(
            drain_inst.ins, ScopedClock({None: tick_clock.global_clock}))
        si = drain_inst.ins.sync_info
        waits = list(si.on_wait or [])
        if len(waits) > 1:
            si.on_wait = waits[:1]
            for i, w in enumerate(waits[1:]):
                nop = self.nc.sync.nop(nofuse=True, hint=f"tailwait{i}")
                nop.ins.sync_info = mybir.SyncInfo(on_wait=[w], on_update=[])
        self.nc.all_engine_barrier()
        popped = self.nc._tile_sem_poison_stack.pop()
        assert popped is self._sem_poison
        self.nc.clear_and_free_semaphores(list(self.sems.allocated().values()))
        self.nc.all_engine_barrier()

    TileContext._drain_and_barrier = _patched
    TileContext._attnpool_patched = True


def _split_multi_waits(nc):
    """Post-pass: hoist extra sem waits onto single-wait NOPs."""
    from concourse import mybir
    n = 0
    for f in nc.m.functions:
        for blk in f.blocks:
            new = []
            for inst in blk.instructions:
                si = inst.sync_info
                waits = list(si.on_wait or []) if si else []
                if len(waits) > 1:
                    for w in waits[:-1]:
                        n += 1
                        nop = mybir.InstNoOp(name=f"I-waitsplit{n}", ins=[], outs=[])
                        nop.engine = inst.engine
                        nop.sync_info = mybir.SyncInfo(on_wait=[w], on_update=[])
                        new.append(nop)
                    si.on_wait = waits[-1:]
                new.append(inst)
            blk.instructions = new


# ---------------------------------------------------------------------------
# device program
# ---------------------------------------------------------------------------

def _build_kernel(NT, windows, b2_plus_25):
    from concourse import bass, mybir
    import concourse.tile as tile
    from concourse import hw_specs

    # The scheduler's cost model under-prices ScalarE/VectorE ops ~2x vs
    # measured HW (the 2x 16-bit accel never applies to fp32-PSUM reads,
    # and per-op overhead is higher than modeled), so it schedules their
    # results just-in-time and the real PE stalls on them every group.
    # Inflate the modeled access cost to measured reality so the
    # scheduler builds in the right slack.
    _ac = dict(hw_specs.TRN2Spec.ACCESS_CYCLES)
    _ac[(bass.MemorySpace.SBUF, mybir.EngineType.Activation)] = 460
    _ac[(bass.MemorySpace.PSUM, mybir.EngineType.Activation)] = 570
    _ac[(bass.MemorySpace.SBUF, mybir.EngineType.DVE)] = 150
    _ac[(bass.MemorySpace.PSUM, mybir.EngineType.DVE)] = 250
    hw_specs.TRN2Spec.ACCESS_CYCLES = _ac

    f32 = mybir.dt.float32
    fp16 = mybir.dt.float16

    nc = bass.Bass()
    NW = len(windows)
    NG = NT // GRP

    fp8 = mybir.dt.float8e4
    x_in = nc.declare_dram_parameter("x", [NG, P, GRP * DIM], fp16, isOutput=False)
    xT_in = nc.declare_dram_parameter("xT", [NG, P, NCH * GRP * P], fp8, isOutput=False)
    rel_in = nc.declare_dram_parameter("rel", [P, NT], f32, isOutput=False)
    w1_in = nc.declare_dram_parameter("w1", [DIM, HID], fp8, isOutput=False)
    w2c_in = nc.declare_dram_parameter("w2c", [P, NHH], f32, isOutput=False)
    b1_in = nc.declare_dram_parameter("b1", [HID], f32, isOutput=False)
    iota_in = nc.declare_dram_parameter("iota", [P, P], fp16, isOutput=False)
    ones_in = nc.declare_dram_parameter("ones", [P, 1], fp16, isOutput=False)
    neg25_in = nc.declare_dram_parameter("neg25", [P, 1], f32, isOutput=False)
    u_out = nc.declare_dram_parameter("u", [NW, P, DIM], f32, isOutput=True)
    s_out = nc.declare_dram_parameter("s", [P, NT], f32, isOutput=True)

    win_start = {a: w for w, (a, b) in enumerate(windows)}
    win_end = {b - 1: w for w, (a, b) in enumerate(windows)}
    tile_win = {}
    for w, (a, b) in enumerate(windows):
        for t in range(a, b):
            tile_win[t] = w

    with tile.TileContext(nc) as tc:
        with tc.tile_pool(name="const", bufs=1) as const, \
             tc.tile_pool(name="xpool", bufs=9) as xpool, \
             tc.tile_pool(name="xtpool", bufs=4) as xtpool, \
             tc.tile_pool(name="thpool", bufs=4) as thpool, \
             tc.tile_pool(name="mpool", bufs=4) as mpool, \
             tc.tile_pool(name="apool", bufs=6) as apool, \
             tc.tile_pool(name="mgpool", bufs=4) as mgpool, \
             tc.tile_pool(name="spool", bufs=3) as spool, \
             tc.tile_pool(name="opool", bufs=2) as opool, \
             tc.tile_pool(name="pp_h", bufs=4, space="PSUM") as pp_h, \
             tc.tile_pool(name="pp_s", bufs=2, space="PSUM") as pp_s, \
             tc.tile_pool(name="pp_u", bufs=1, space="PSUM") as pp_u:

            w1t = const.tile([P, NCH, HID], fp8)
            nc.sync.dma_start(out=w1t[:], in_=w1_in.rearrange("(c p) h -> p c h", p=P))
            w2c = const.tile([P, NHH], f32)
            nc.sync.dma_start(out=w2c[:], in_=w2c_in[:])
            b1t = const.tile([P, NHH], f32)
            nc.sync.dma_start(out=b1t[:], in_=b1_in.rearrange("(c p) -> p c", p=P))
            iota = const.tile([P, P], fp16)
            nc.sync.dma_start(out=iota[:], in_=iota_in[:])
            ones = const.tile([P, 1], fp16)
            nc.sync.dma_start(out=ones[:], in_=ones_in[:])
            neg25 = const.tile([P, 1], f32)
            nc.sync.dma_start(out=neg25[:], in_=neg25_in[:])
            relt = const.tile([P, NT], f32)
            nc.sync.dma_start(out=relt[:], in_=rel_in[:])
            s_stage = const.tile([P, NT], f32)

            state = {}
            ugroups = {}

            def emit_mlp(g):
                xg = xpool.tile([P, GRP, DIM], fp16, tag="xg")
                nc.sync.dma_start(out=xg[:].rearrange("p t d -> p (t d)"), in_=x_in[g])
                xTg = xtpool.tile([P, NCH, GRP * P], fp8, tag="xTg")
                nc.sync.dma_start(out=xTg[:].rearrange("p c n -> p (c n)"), in_=xT_in[g])

                thT = thpool.tile([P, NHH, GRP * P], fp16, tag="thT")
                for hh in range(NHH):
                    hTp = pp_h.tile([P, GRP * P], f32, tag="hT", name=f"hTp{hh}")
                    for c in range(0, NCH, 2):
                        nc.tensor.matmul(
                            hTp[:],
                            lhsT=w1t[:, c:c + 2, hh * P:(hh + 1) * P],
                            rhs=xTg[:, c:c + 2],
                            start=(c == 0), stop=(c == NCH - 2),
                            perf_mode=mybir.MatmulPerfMode.DoubleRow)
                    nc.scalar.activation(
                        thT[:, hh], hTp[:],
                        mybir.ActivationFunctionType.Tanh,
                        bias=b1t[:, hh:hh + 1])
                state[g] = {"xg": xg, "thT": thT}

            def emit_score1(g):
                t0 = g * GRP
                st = state[g]
                ctx = tc.high_priority()
                ctx.__enter__()
                # one-hot masks (constants only -> never blocks the DVE queue)
                mg = mgpool.tile([P, GRP, P], fp16, tag="mg")
                for i in range(GRP):
                    nc.vector.tensor_scalar(
                        out=mg[:, i],
                        in0=iota[:],
                        scalar1=relt[:, t0 + i:t0 + i + 1],
                        scalar2=float(b2_plus_25),
                        op0=mybir.AluOpType.is_equal,
                        op1=mybir.AluOpType.mult)
                # mm[p, n] = w2[p]*th0[p, n] + w2[128+p]*th1[p, n]
                m0 = mpool.tile([P, GRP * P], fp16, tag="m0")
                nc.vector.tensor_scalar(
                    out=m0[:], in0=st["thT"][:, 0], scalar1=w2c[:, 0:1],
                    scalar2=None, op0=mybir.AluOpType.mult)
                mm = mpool.tile([P, GRP * P], fp16, tag="mm")
                nc.vector.scalar_tensor_tensor(
                    out=mm[:], in0=st["thT"][:, 1], scalar=w2c[:, 1:2],
                    in1=m0[:], op0=mybir.AluOpType.mult,
                    op1=mybir.AluOpType.add)
                st["mg"] = mg
                st["mm"] = mm
                ctx.__exit__(None, None, None)

            def emit_colsum(g):
                st = state[g]
                # s per node, transposed onto partitions via colsum matmul
                sp = pp_s.tile([P, GRP], f32, tag="sp")
                for i in range(GRP):
                    nc.tensor.matmul(
                        sp[:, i:i + 1],
                        lhsT=st["mm"][:, i * P:(i + 1) * P],
                        rhs=ones[:],
                        start=True, stop=True)
                st["sp"] = sp

            def emit_score2(g):
                t0 = g * GRP
                st = state[g]
                ctx = tc.high_priority()
                ctx.__enter__()
                # s-25 lands in the persistent staging tile (shipped to host,
                # which rebuilds e = exp(s) and the denominators from it) and
                # doubles as the per-tile exp bias. On ScalarE: keeps the DVE
                # queue free of ops that wait on same-iteration PE results.
                nc.scalar.activation(
                    s_stage[:, t0:t0 + GRP], st["sp"][:],
                    mybir.ActivationFunctionType.Identity,
                    bias=neg25[:, 0:1])
                A = apool.tile([P, GRP, P], fp16, tag="A")
                for i in range(GRP):
                    nc.scalar.activation(
                        A[:, i], st["mg"][:, i],
                        mybir.ActivationFunctionType.Exp,
                        bias=s_stage[:, t0 + i:t0 + i + 1])
                st["A"] = A
                ctx.__exit__(None, None, None)

            def emit_scatter(g):
                ts = [g * GRP + i for i in range(GRP)]
                st = state[g]
                for i, t in enumerate(ts):
                    xt = st["xg"][:, i]
                    w = tile_win[t]
                    if t in win_start:
                        uwin = pp_u.tile([P, DIM], f32, tag="uwin")
                        ugroups[w] = uwin
                    up = ugroups[w]
                    for half in range(2):
                        nc.tensor.matmul(
                            up[:, half * 512:(half + 1) * 512],
                            lhsT=st["A"][:, i],
                            rhs=xt[:, half * 512:(half + 1) * 512],
                            start=(t in win_start), stop=(t in win_end))
                    if t in win_end:
                        us = opool.tile([P, DIM], f32, tag="us")
                        nc.vector.tensor_copy(us[:, 0:512], up[:, 0:512])
                        nc.scalar.copy(us[:, 512:1024], up[:, 512:1024])
                        nc.sync.dma_start(out=u_out[w], in_=us[:])
                del state[g]

            # per iteration g: every PE input is >=1 full iteration old.
            #   Scalar: sb(g-3)+exp(g-3) first, then tanh(g)
            #   PE:  mlp(g), colsum(g-2), scatter(g-5)
            #   DVE: mg/m0/mm(g-1), e-reduce(g-5)
            for g in range(NG + 6):
                if 3 <= g < NG + 3:
                    emit_score2(g - 3)
                if g < NG:
                    emit_mlp(g)
                if 1 <= g < NG + 1:
                    emit_score1(g - 1)
                if 2 <= g < NG + 2:
                    emit_colsum(g - 2)
                if 6 <= g:
                    emit_scatter(g - 6)

            nc.sync.dma_start(out=s_out[:], in_=s_stage[:])

    return nc


# ---------------------------------------------------------------------------
# host wrapper
# ---------------------------------------------------------------------------

def _make_windows(NT, nw):
    base, rem = divmod(NT, nw)
    sizes = [base + (1 if i < rem else 0) for i in range(nw)]
    out, a = [], 0
    for s in sizes:
        out.append((a, a + s))
        a += s
    return out


def _reference_numpy(x, batch, W1, b1, W2, b2):
    """Fallback for inputs outside this kernel's structural assumptions."""
    h = np.tanh(x.astype(np.float64) @ W1.astype(np.float64) + b1)
    s = (h @ W2.astype(np.float64) + b2).ravel()
    e = np.exp(s - s.max())
    denom = np.zeros(NUM_SEG, dtype=np.float64)
    np.add.at(denom, batch, e)
    attn = e / (denom[batch] + 1e-8)
    out = np.zeros((NUM_SEG, x.shape[1]), dtype=np.float64)
    np.add.at(out, batch, attn[:, None] * x.astype(np.float64))
    return out.astype(np.float32)


def kernel(x, batch, W1, b1, W2, b2):
    x = np.ascontiguousarray(np.asarray(x, dtype=np.float32))
    batch64 = np.asarray(batch).astype(np.int64)
    W1 = np.asarray(W1, dtype=np.float32)
    b1 = np.asarray(b1, dtype=np.float32)
    W2 = np.asarray(W2, dtype=np.float32)
    b2 = np.asarray(b2, dtype=np.float32)

    N = x.shape[0]
    pc = N // N_CORES
    NT = pc // P
    NG = NT // GRP if NT else 0

    ok = (N == N_CORES * pc and pc == NT * P and NT % GRP == 0
          and x.shape[1] == DIM and W1.shape == (DIM, HID)
          and np.all(batch64[:-1] <= batch64[1:])
          and batch64.min() >= 0 and batch64.max() < NUM_SEG)
    if not ok:
        return _reference_numpy(x, batch64, W1, b1, W2, b2)

    windows = _make_windows(NT, 3)
    iota = np.tile(np.arange(P, dtype=np.float16), (P, 1))
    b2f = float(b2.reshape(-1)[0])
    w2col = np.ascontiguousarray(
        W2.reshape(NHH, P).T.astype(np.float32))          # [P, NHH]
    ones16 = np.ones((P, 1), dtype=np.float16)

    in_maps, meta = [], []
    for c in range(N_CORES):
        bb = batch64[c * pc:(c + 1) * pc]
        rel = np.empty((NT, P), dtype=np.float32)
        bases = []
        for w, (a, b) in enumerate(windows):
            base = int(bb[a * P])
            bases.append(base)
            seg_rel = bb[a * P:b * P] - base
            if seg_rel.min() < 0 or seg_rel.max() >= P:
                return _reference_numpy(x, batch64, W1, b1, W2, b2)
            rel[a:b] = seg_rel.reshape(b - a, P).astype(np.float32)
        xc = x[c * pc:(c + 1) * pc]
        x16 = xc.astype(np.float16)
        x8 = xc.astype(ml_dtypes.float8_e4m3)
        xt = np.ascontiguousarray(
            x16.reshape(NG, GRP, P, DIM).transpose(0, 2, 1, 3)
               .reshape(NG, P, GRP * DIM))
        xTt = np.ascontiguousarray(
            x8.reshape(NG, GRP * P, NCH, P).transpose(0, 3, 2, 1)
               .reshape(NG, P, NCH * GRP * P))
        in_maps.append({
            "x": xt,
            "xT": xTt,
            "rel": np.ascontiguousarray(rel.T),
            "w1": W1.astype(ml_dtypes.float8_e4m3),
            "w2c": w2col,
            "b1": b1,
            "iota": iota,
            "ones": ones16,
            "neg25": np.full((P, 1), -25.0, dtype=np.float32),
        })
        meta.append(bases)

    _install_ntff_hook()
    _install_tile_compat()
    from concourse.bass_utils import run_bass_kernel_spmd

    nc = _build_kernel(NT, windows, b2f + 25.0)
    _split_multi_waits(nc)

    trace = os.environ.get("KERNEL_TRACE", "") == "1"
    res = run_bass_kernel_spmd(nc, in_maps, list(range(N_CORES)), trace=trace)
    if trace and res.exec_time_ns:
        print(f"[kernel] HW exec time: {res.exec_time_ns} ns", file=sys.stderr)
        kernel.last_exec_time_ns = res.exec_time_ns

    # host unshard: accumulate windows, build denominators from e, divide
    u_sum = np.zeros((NUM_SEG, DIM), dtype=np.float64)
    e_full = np.empty(N, dtype=np.float64)
    for c in range(N_CORES):
        r = res.results[c]
        s25 = r["s"].T.reshape(-1).astype(np.float64)
        e_full[c * pc:(c + 1) * pc] = np.exp(
            s25 + 25.0 + np.float64(np.float16(b2f + 25.0)) - 25.0)
        for w in range(len(windows)):
            base = meta[c][w]
            hi = min(base + P, NUM_SEG)
            u_sum[base:hi] += r["u"][w][:hi - base]
    denom = np.zeros(NUM_SEG, dtype=np.float64)
    np.add.at(denom, batch64, e_full)
    s_max = float(np.log(max(e_full.max(), 1e-30)))
    out = u_sum / (denom + 1e-8 * np.exp(s_max))[:, None]
    return out.astype(np.float32)


kernel.last_exec_time_ns = None


# revision 23
# speedup vs baseline: 1.1823x; 1.0924x over previous
"""nn_AttentionPool Trainium2 kernel.

kernel(x, batch, W1, b1, W2, b2) -> np.ndarray [2048, 1024] float32

Strategy (8 NeuronCores, SPMD, data-parallel over node rows; batch is
sorted so each core covers a contiguous segment range):
  - Host ships per core: x in fp16 twice, pre-tiled for contiguous DMA —
    node-major (scatter operand) and dim-major (MLP moving operand).
  - Per 4-tile group (512 nodes) on device (3-stage pipeline, staggered
    2 groups per stage so PE never waits on ACT/DVE latency):
      stage A (mlp):   PE  hT[hh] += W1[:,c,hh]^T @ xT[:,c]  (fp16, N=512)
                       ACT thT = tanh(hT + b1) -> fp16
      stage B (score): DVE mm = th0*w2c0 + th1*w2c1          (fp16)
                       PE  s[i]  = mm_slice^T @ ones          (colsum = per-
                            node score, transposed onto partitions)
                       DVE sb = s - 25
                       DVE mg[i] = (iota==rel)*(b2+25)        (fp16 one-hot)
                       ACT A[i] = exp(mg + sb), accum_out -> e[:,t]
                            (match -> exp(s+b2); miss -> exp(s-25) ~ 0)
      stage C (scatter): PE u_win += A^T @ x  (fp16, 2x N=512, static
                            node-window PSUM accumulators; 3 windows/core)
  - Host: accumulates window outputs by true segment base, builds
    denominators from e, divides (reference epsilon semantics).
Max-shift note: s in [-1.2, 1.2] for this model so unshifted exp is safe;
softmax normalization cancels any constant shift.
"""
import os
import sys
import types

import ml_dtypes
import numpy as np

P = 128
DIM = 1024
HID = 256
NCH = DIM // P
NHH = HID // P
GRP = 4
N_CORES = 8
NUM_SEG = 2048

# ---------------------------------------------------------------------------
# environment compat (axon-tunneled trn2 + this walrus build)
# ---------------------------------------------------------------------------

def _install_ntff_hook():
    """antenv.axon_hooks is absent in this image; reconstruct it so
    trace=True (KERNEL_TRACE=1) can profile. Harmless if unused."""
    if "antenv.axon_hooks" in sys.modules:
        return
    m = types.ModuleType("antenv.axon_hooks")
    m._hook = None
    m.set_axon_ntff_profile_hook = lambda h: setattr(m, "_hook", h)
    m.get_axon_ntff_profile_hook = lambda: m._hook
    sys.modules["antenv.axon_hooks"] = m
    try:
        from trn_agent_boot.trn_boot import _ntff_profile_via_ctypes
        m.set_axon_ntff_profile_hook(
            _ntff_profile_via_ctypes("/opt/axon/libaxon_pjrt.so"))
    except Exception:
        pass


def _install_tile_compat():
    """This walrus accepts at most ONE sem wait per instruction; Tile's exit
    drain carries one per live proc. Patch the drain to spread waits."""
    from concourse import mybir
    from concourse.tile import TileContext, ScopedClock

    if getattr(TileContext, "_attnpool_patched", False):
        return

    def _patched(self, tick_clock, wait_clock):
        drain_inst = self.nc.sync.drain()
        wait_clock.add_sem_waits(
            drain_inst.ins, ScopedClock({None: tick_clock.global_clock}))
        si = drain_inst.ins.sync_info
        waits = list(si.on_wait or [])
        if len(waits) > 1:
            si.on_wait = waits[:1]
            for i, w in enumerate(waits[1:]):
                nop = self.nc.sync.nop(nofuse=True, hint=f"tailwait{i}")
                nop.ins.sync_info = mybir.SyncInfo(on_wait=[w], on_update=[])
        self.nc.all_engine_barrier()
        popped = self.nc._tile_sem_poison_stack.pop()
        assert popped is self._sem_poison
        self.nc.clear_and_free_semaphores(list(self.sems.allocated().values()))
        self.nc.all_engine_barrier()

    TileContext._drain_and_barrier = _patched
    TileContext._attnpool_patched = True


def _split_multi_waits(nc):
    """Post-pass: hoist extra sem waits onto single-wait NOPs."""
    from concourse import mybir
    n = 0
    for f in nc.m.functions:
        for blk in f.blocks:
            new = []
            for inst in blk.instructions:
                si = inst.sync_info
                waits = list(si.on_wait or []) if si else []
                if len(waits) > 1:
                    for w in waits[:-1]:
                        n += 1
                        nop = mybir.InstNoOp(name=f"I-waitsplit{n}", ins=[], outs=[])
                        nop.engine = inst.engine
                        nop.sync_info = mybir.SyncInfo(on_wait=[w], on_update=[])
                        new.append(nop)
                    si.on_wait = waits[-1:]
                new.append(inst)
            blk.instructions = new


# ---------------------------------------------------------------------------
# device program
# ---------------------------------------------------------------------------

def _build_kernel(NT, windows, b2_plus_25):
    from concourse import bass, mybir
    import concourse.tile as tile
    from concourse import hw_specs

    # The scheduler's cost model under-prices ScalarE/VectorE ops ~2x vs
    # measured HW (the 2x 16-bit accel never applies to fp32-PSUM reads,
    # and per-op overhead is higher than modeled), so it schedules their
    # results just-in-time and the real PE stalls on them every group.
    # Inflate the modeled access cost to measured reality so the
    # scheduler builds in the right slack.
    _ac = dict(hw_specs.TRN2Spec.ACCESS_CYCLES)
    _ac[(bass.MemorySpace.SBUF, mybir.EngineType.Activation)] = 460
    _ac[(bass.MemorySpace.PSUM, mybir.EngineType.Activation)] = 570
    _ac[(bass.MemorySpace.SBUF, mybir.EngineType.DVE)] = 150
    _ac[(bass.MemorySpace.PSUM, mybir.EngineType.DVE)] = 250
    hw_specs.TRN2Spec.ACCESS_CYCLES = _ac

    f32 = mybir.dt.float32
    fp16 = mybir.dt.float16

    nc = bass.Bass()
    NW = len(windows)
    NG = NT // GRP

    fp8 = mybir.dt.float8e4
    x_in = nc.declare_dram_parameter("x", [NG, P, GRP * DIM], fp16, isOutput=False)
    xT_in = nc.declare_dram_parameter("xT", [NG, P, NCH * GRP * P], fp8, isOutput=False)
    rel_in = nc.declare_dram_parameter("rel", [P, NT], f32, isOutput=False)
    w1_in = nc.declare_dram_parameter("w1", [DIM, HID], fp8, isOutput=False)
    w2c_in = nc.declare_dram_parameter("w2c", [P, NHH], f32, isOutput=False)
    b1_in = nc.declare_dram_parameter("b1", [HID], f32, isOutput=False)
    iota_in = nc.declare_dram_parameter("iota", [P, P], fp16, isOutput=False)
    ones_in = nc.declare_dram_parameter("ones", [P, 1], fp16, isOutput=False)
    neg25_in = nc.declare_dram_parameter("neg25", [P, 1], f32, isOutput=False)
    u_out = nc.declare_dram_parameter("u", [NW, P, DIM], f32, isOutput=True)
    s_out = nc.declare_dram_parameter("s", [P, NT], f32, isOutput=True)

    win_start = {a: w for w, (a, b) in enumerate(windows)}
    win_end = {b - 1: w for w, (a, b) in enumerate(windows)}
    tile_win = {}
    for w, (a, b) in enumerate(windows):
        for t in range(a, b):
            tile_win[t] = w

    with tile.TileContext(nc) as tc:
        with tc.tile_pool(name="const", bufs=1) as const, \
             tc.tile_pool(name="xpool", bufs=8) as xpool, \
             tc.tile_pool(name="xtpool", bufs=4) as xtpool, \
             tc.tile_pool(name="thpool", bufs=4) as thpool, \
             tc.tile_pool(name="mpool", bufs=4) as mpool, \
             tc.tile_pool(name="apool", bufs=4) as apool, \
             tc.tile_pool(name="mgpool", bufs=4) as mgpool, \
             tc.tile_pool(name="spool", bufs=3) as spool, \
             tc.tile_pool(name="opool", bufs=2) as opool, \
             tc.tile_pool(name="pp_h", bufs=4, space="PSUM") as pp_h, \
             tc.tile_pool(name="pp_s", bufs=2, space="PSUM") as pp_s, \
             tc.tile_pool(name="pp_u", bufs=1, space="PSUM") as pp_u:

            w1t = const.tile([P, NCH, HID], fp8)
            nc.sync.dma_start(out=w1t[:], in_=w1_in.rearrange("(c p) h -> p c h", p=P))
            w2c = const.tile([P, NHH], f32)
            nc.sync.dma_start(out=w2c[:], in_=w2c_in[:])
            b1t = const.tile([P, NHH], f32)
            nc.sync.dma_start(out=b1t[:], in_=b1_in.rearrange("(c p) -> p c", p=P))
            iota = const.tile([P, P], fp16)
            nc.sync.dma_start(out=iota[:], in_=iota_in[:])
            ones = const.tile([P, 1], fp16)
            nc.sync.dma_start(out=ones[:], in_=ones_in[:])
            neg25 = const.tile([P, 1], f32)
            nc.sync.dma_start(out=neg25[:], in_=neg25_in[:])
            relt = const.tile([P, NT], f32)
            nc.sync.dma_start(out=relt[:], in_=rel_in[:])
            s_stage = const.tile([P, NT], f32)

            state = {}
            ugroups = {}

            def emit_mlp(g):
                xg = xpool.tile([P, GRP, DIM], fp16, tag="xg")
                nc.sync.dma_start(out=xg[:].rearrange("p t d -> p (t d)"), in_=x_in[g])
                xTg = xtpool.tile([P, NCH, GRP * P], fp8, tag="xTg")
                nc.sync.dma_start(out=xTg[:].rearrange("p c n -> p (c n)"), in_=xT_in[g])

                thT = thpool.tile([P, NHH, GRP * P], fp16, tag="thT")
                for hh in range(NHH):
                    hTp = pp_h.tile([P, GRP * P], f32, tag="hT", name=f"hTp{hh}")
                    for c in range(0, NCH, 2):
                        nc.tensor.matmul(
                            hTp[:],
                            lhsT=w1t[:, c:c + 2, hh * P:(hh + 1) * P],
                            rhs=xTg[:, c:c + 2],
                            start=(c == 0), stop=(c == NCH - 2),
                            perf_mode=mybir.MatmulPerfMode.DoubleRow)
                    nc.scalar.activation(
                        thT[:, hh], hTp[:],
                        mybir.ActivationFunctionType.Tanh,
                        bias=b1t[:, hh:hh + 1])
                state[g] = {"xg": xg, "thT": thT}

            def emit_score1(g):
                t0 = g * GRP
                st = state[g]
                ctx = tc.high_priority()
                ctx.__enter__()
                # one-hot masks (constants only -> never blocks the DVE queue)
                mg = mgpool.tile([P, GRP, P], fp16, tag="mg")
                for i in range(GRP):
                    nc.vector.tensor_scalar(
                        out=mg[:, i],
                        in0=iota[:],
                        scalar1=relt[:, t0 + i:t0 + i + 1],
                        scalar2=float(b2_plus_25),
                        op0=mybir.AluOpType.is_equal,
                        op1=mybir.AluOpType.mult)
                # mm[p, n] = w2[p]*th0[p, n] + w2[128+p]*th1[p, n]
                m0 = mpool.tile([P, GRP * P], fp16, tag="m0")
                nc.vector.tensor_scalar(
                    out=m0[:], in0=st["thT"][:, 0], scalar1=w2c[:, 0:1],
                    scalar2=None, op0=mybir.AluOpType.mult)
                mm = mpool.tile([P, GRP * P], fp16, tag="mm")
                nc.vector.scalar_tensor_tensor(
                    out=mm[:], in0=st["thT"][:, 1], scalar=w2c[:, 1:2],
                    in1=m0[:], op0=mybir.AluOpType.mult,
                    op1=mybir.AluOpType.add)
                st["mg"] = mg
                st["mm"] = mm
                ctx.__exit__(None, None, None)

            def emit_colsum(g):
                st = state[g]
                # s per node, transposed onto partitions via colsum matmul
                sp = pp_s.tile([P, GRP], f32, tag="sp")
                for i in range(GRP):
                    nc.tensor.matmul(
                        sp[:, i:i + 1],
                        lhsT=st["mm"][:, i * P:(i + 1) * P],
                        rhs=ones[:],
                        start=True, stop=True)
                st["sp"] = sp

            def emit_score2(g):
                t0 = g * GRP
                st = state[g]
                ctx = tc.high_priority()
                ctx.__enter__()
                # s-25 lands in the persistent staging tile (shipped to host,
                # which rebuilds e = exp(s) and the denominators from it) and
                # doubles as the per-tile exp bias. On ScalarE: keeps the DVE
                # queue free of ops that wait on same-iteration PE results.
                nc.scalar.activation(
                    s_stage[:, t0:t0 + GRP], st["sp"][:],
                    mybir.ActivationFunctionType.Identity,
                    bias=neg25[:, 0:1])
                A = apool.tile([P, GRP, P], fp16, tag="A")
                for i in range(GRP):
                    nc.scalar.activation(
                        A[:, i], st["mg"][:, i],
                        mybir.ActivationFunctionType.Exp,
                        bias=s_stage[:, t0 + i:t0 + i + 1])
                st["A"] = A
                ctx.__exit__(None, None, None)

            def emit_scatter(g):
                ts = [g * GRP + i for i in range(GRP)]
                st = state[g]
                for i, t in enumerate(ts):
                    xt = st["xg"][:, i]
                    w = tile_win[t]
                    if t in win_start:
                        uwin = pp_u.tile([P, DIM], f32, tag="uwin")
                        ugroups[w] = uwin
                    up = ugroups[w]
                    for half in range(2):
                        nc.tensor.matmul(
                            up[:, half * 512:(half + 1) * 512],
                            lhsT=st["A"][:, i],
                            rhs=xt[:, half * 512:(half + 1) * 512],
                            start=(t in win_start), stop=(t in win_end))
                    if t in win_end:
                        us = opool.tile([P, DIM], f32, tag="us")
                        nc.vector.tensor_copy(us[:, 0:512], up[:, 0:512])
                        nc.scalar.copy(us[:, 512:1024], up[:, 512:1024])
                        nc.sync.dma_start(out=u_out[w], in_=us[:])
                del state[g]

            # per iteration g: every PE input is >=1 full iteration old.
            #   Scalar: sb(g-3)+exp(g-3) first, then tanh(g)
            #   PE:  mlp(g), colsum(g-2), scatter(g-5)
            #   DVE: mg/m0/mm(g-1), e-reduce(g-5)
            for g in range(NG + 5):
                if 3 <= g < NG + 3:
                    emit_score2(g - 3)
                if g < NG:
                    emit_mlp(g)
                if 1 <= g < NG + 1:
                    emit_score1(g - 1)
                if 2 <= g < NG + 2:
                    emit_colsum(g - 2)
                if 5 <= g:
                    emit_scatter(g - 5)

            nc.sync.dma_start(out=s_out[:], in_=s_stage[:])

    return nc


# ---------------------------------------------------------------------------
# host wrapper
# ---------------------------------------------------------------------------

def _make_windows(NT, nw):
    base, rem = divmod(NT, nw)
    sizes = [base + (1 if i < rem else 0) for i in range(nw)]
    out, a = [], 0
    for s in sizes:
        out.append((a, a + s))
        a += s
    return out


def _reference_numpy(x, batch, W1, b1, W2, b2):
    """Fallback for inputs outside this kernel's structural assumptions."""
    h = np.tanh(x.astype(np.float64) @ W1.astype(np.float64) + b1)
    s = (h @ W2.astype(np.float64) + b2).ravel()
    e = np.exp(s - s.max())
    denom = np.zeros(NUM_SEG, dtype=np.float64)
    np.add.at(denom, batch, e)
    attn = e / (denom[batch] + 1e-8)
    out = np.zeros((NUM_SEG, x.shape[1]), dtype=np.float64)
    np.add.at(out, batch, attn[:, None] * x.astype(np.float64))
    return out.astype(np.float32)


def kernel(x, batch, W1, b1, W2, b2):
    x = np.ascontiguousarray(np.asarray(x, dtype=np.float32))
    batch64 = np.asarray(batch).astype(np.int64)
    W1 = np.asarray(W1, dtype=np.float32)
    b1 = np.asarray(b1, dtype=np.float32)
    W2 = np.asarray(W2, dtype=np.float32)
    b2 = np.asarray(b2, dtype=np.float32)

    N = x.shape[0]
    pc = N // N_CORES
    NT = pc // P
    NG = NT // GRP if NT else 0

    ok = (N == N_CORES * pc and pc == NT * P and NT % GRP == 0
          and x.shape[1] == DIM and W1.shape == (DIM, HID)
          and np.all(batch64[:-1] <= batch64[1:])
          and batch64.min() >= 0 and batch64.max() < NUM_SEG)
    if not ok:
        return _reference_numpy(x, batch64, W1, b1, W2, b2)

    windows = _make_windows(NT, 3)
    iota = np.tile(np.arange(P, dtype=np.float16), (P, 1))
    b2f = float(b2.reshape(-1)[0])
    w2col = np.ascontiguousarray(
        W2.reshape(NHH, P).T.astype(np.float32))          # [P, NHH]
    ones16 = np.ones((P, 1), dtype=np.float16)

    in_maps, meta = [], []
    for c in range(N_CORES):
        bb = batch64[c * pc:(c + 1) * pc]
        rel = np.empty((NT, P), dtype=np.float32)
        bases = []
        for w, (a, b) in enumerate(windows):
            base = int(bb[a * P])
            bases.append(base)
            seg_rel = bb[a * P:b * P] - base
            if seg_rel.min() < 0 or seg_rel.max() >= P:
                return _reference_numpy(x, batch64, W1, b1, W2, b2)
            rel[a:b] = seg_rel.reshape(b - a, P).astype(np.float32)
        xc = x[c * pc:(c + 1) * pc]
        x16 = xc.astype(np.float16)
        x8 = xc.astype(ml_dtypes.float8_e4m3)
        xt = np.ascontiguousarray(
            x16.reshape(NG, GRP, P, DIM).transpose(0, 2, 1, 3)
               .reshape(NG, P, GRP * DIM))
        xTt = np.ascontiguousarray(
            x8.reshape(NG, GRP * P, NCH, P).transpose(0, 3, 2, 1)
               .reshape(NG, P, NCH * GRP * P))
        in_maps.append({
            "x": xt,
            "xT": xTt,
            "rel": np.ascontiguousarray(rel.T),
            "w1": W1.astype(ml_dtypes.float8_e4m3),
            "w2c": w2col,
            "b1": b1,
            "iota": iota,
            "ones": ones16,
            "neg25": np.full((P, 1), -25.0, dtype=np.float32),
        })
        meta.append(bases)

    _install_ntff_hook()
    _install_tile_compat()
    from concourse.bass_utils import run_bass_kernel_spmd

    nc = _build_kernel(NT, windows, b2f + 25.0)
    _split_multi_waits(nc)

    trace = os.environ.get("KERNEL_TRACE", "") == "1"
    res = run_bass_kernel_spmd(nc, in_maps, list(range(N_CORES)), trace=trace)
    if trace and res.exec_time_ns:
        print(f"[kernel] HW exec time: {res.exec_time_ns} ns", file=sys.stderr)
        kernel.last_exec_time_ns = res.exec_time_ns

    # host unshard: accumulate windows, build denominators from e, divide
    u_sum = np.zeros((NUM_SEG, DIM), dtype=np.float64)
    e_full = np.empty(N, dtype=np.float64)
    for c in range(N_CORES):
        r = res.results[c]
        s25 = r["s"].T.reshape(-1).astype(np.float64)
        e_full[c * pc:(c + 1) * pc] = np.exp(
            s25 + 25.0 + np.float64(np.float16(b2f + 25.0)) - 25.0)
        for w in range(len(windows)):
            base = meta[c][w]
            hi = min(base + P, NUM_SEG)
            u_sum[base:hi] += r["u"][w][:hi - base]
    denom = np.zeros(NUM_SEG, dtype=np.float64)
    np.add.at(denom, batch64, e_full)
    s_max = float(np.log(max(e_full.max(), 1e-30)))
    out = u_sum / (denom + 1e-8 * np.exp(s_max))[:, None]
    return out.astype(np.float32)


kernel.last_exec_time_ns = None


# revision 24
# speedup vs baseline: 1.2925x; 1.0931x over previous
"""nn_AttentionPool Trainium2 kernel.

kernel(x, batch, W1, b1, W2, b2) -> np.ndarray [2048, 1024] float32

Strategy (8 NeuronCores, SPMD, data-parallel over node rows; batch is
sorted so each core covers a contiguous segment range):
  - Host ships per core: x in fp16 twice, pre-tiled for contiguous DMA —
    node-major (scatter operand) and dim-major (MLP moving operand).
  - Per 4-tile group (512 nodes) on device (3-stage pipeline, staggered
    2 groups per stage so PE never waits on ACT/DVE latency):
      stage A (mlp):   PE  hT[hh] += W1[:,c,hh]^T @ xT[:,c]  (fp16, N=512)
                       ACT thT = tanh(hT + b1) -> fp16
      stage B (score): DVE mm = th0*w2c0 + th1*w2c1          (fp16)
                       PE  s[i]  = mm_slice^T @ ones          (colsum = per-
                            node score, transposed onto partitions)
                       DVE sb = s - 25
                       DVE mg[i] = (iota==rel)*(b2+25)        (fp16 one-hot)
                       ACT A[i] = exp(mg + sb), accum_out -> e[:,t]
                            (match -> exp(s+b2); miss -> exp(s-25) ~ 0)
      stage C (scatter): PE u_win += A^T @ x  (fp16, 2x N=512, static
                            node-window PSUM accumulators; 3 windows/core)
  - Host: accumulates window outputs by true segment base, builds
    denominators from e, divides (reference epsilon semantics).
Max-shift note: s in [-1.2, 1.2] for this model so unshifted exp is safe;
softmax normalization cancels any constant shift.
"""
import os
import sys
import types

import ml_dtypes
import numpy as np

P = 128
DIM = 1024
HID = 256
NCH = DIM // P
NHH = HID // P
GRP = 4
N_CORES = 8
NUM_SEG = 2048

# ---------------------------------------------------------------------------
# environment compat (axon-tunneled trn2 + this walrus build)
# ---------------------------------------------------------------------------

def _install_ntff_hook():
    """antenv.axon_hooks is absent in this image; reconstruct it so
    trace=True (KERNEL_TRACE=1) can profile. Harmless if unused."""
    if "antenv.axon_hooks" in sys.modules:
        return
    m = types.ModuleType("antenv.axon_hooks")
    m._hook = None
    m.set_axon_ntff_profile_hook = lambda h: setattr(m, "_hook", h)
    m.get_axon_ntff_profile_hook = lambda: m._hook
    sys.modules["antenv.axon_hooks"] = m
    try:
        from trn_agent_boot.trn_boot import _ntff_profile_via_ctypes
        m.set_axon_ntff_profile_hook(
            _ntff_profile_via_ctypes("/opt/axon/libaxon_pjrt.so"))
    except Exception:
        pass


def _install_tile_compat():
    """This walrus accepts at most ONE sem wait per instruction; Tile's exit
    drain carries one per live proc. Patch the drain to spread waits."""
    from concourse import mybir
    from concourse.tile import TileContext, ScopedClock

    if getattr(TileContext, "_attnpool_patched", False):
        return

    def _patched(self, tick_clock, wait_clock):
        drain_inst = self.nc.sync.drain()
        wait_clock.add_sem_waits(
            drain_inst.ins, ScopedClock({None: tick_clock.global_clock}))
        si = drain_inst.ins.sync_info
        waits = list(si.on_wait or [])
        if len(waits) > 1:
            si.on_wait = waits[:1]
            for i, w in enumerate(waits[1:]):
                nop = self.nc.sync.nop(nofuse=True, hint=f"tailwait{i}")
                nop.ins.sync_info = mybir.SyncInfo(on_wait=[w], on_update=[])
        self.nc.all_engine_barrier()
        popped = self.nc._tile_sem_poison_stack.pop()
        assert popped is self._sem_poison
        self.nc.clear_and_free_semaphores(list(self.sems.allocated().values()))
        self.nc.all_engine_barrier()

    TileContext._drain_and_barrier = _patched
    TileContext._attnpool_patched = True


def _split_multi_waits(nc):
    """Post-pass: hoist extra sem waits onto single-wait NOPs."""
    from concourse import mybir
    n = 0
    for f in nc.m.functions:
        for blk in f.blocks:
            new = []
            for inst in blk.instructions:
                si = inst.sync_info
                waits = list(si.on_wait or []) if si else []
                if len(waits) > 1:
                    for w in waits[:-1]:
                        n += 1
                        nop = mybir.InstNoOp(name=f"I-waitsplit{n}", ins=[], outs=[])
                        nop.engine = inst.engine
                        nop.sync_info = mybir.SyncInfo(on_wait=[w], on_update=[])
                        new.append(nop)
                    si.on_wait = waits[-1:]
                new.append(inst)
            blk.instructions = new


# ---------------------------------------------------------------------------
# device program
# ---------------------------------------------------------------------------

def _build_kernel(NT, windows, b2_plus_25):
    from concourse import bass, mybir
    import concourse.tile as tile
    from concourse import hw_specs

    # The scheduler's cost model under-prices ScalarE/VectorE ops ~2x vs
    # measured HW (the 2x 16-bit accel never applies to fp32-PSUM reads,
    # and per-op overhead is higher than modeled), so it schedules their
    # results just-in-time and the real PE stalls on them every group.
    # Inflate the modeled access cost to measured reality so the
    # scheduler builds in the right slack.
    _ac = dict(hw_specs.TRN2Spec.ACCESS_CYCLES)
    _ac[(bass.MemorySpace.SBUF, mybir.EngineType.Activation)] = 460
    _ac[(bass.MemorySpace.PSUM, mybir.EngineType.Activation)] = 570
    _ac[(bass.MemorySpace.SBUF, mybir.EngineType.DVE)] = 150
    _ac[(bass.MemorySpace.PSUM, mybir.EngineType.DVE)] = 250
    hw_specs.TRN2Spec.ACCESS_CYCLES = _ac

    f32 = mybir.dt.float32
    fp16 = mybir.dt.float16

    nc = bass.Bass()
    NW = len(windows)
    NG = NT // GRP

    fp8 = mybir.dt.float8e4
    x_in = nc.declare_dram_parameter("x", [NG, P, GRP * DIM], fp16, isOutput=False)
    xT_in = nc.declare_dram_parameter("xT", [NG, P, NCH * GRP * P], fp8, isOutput=False)
    rel_in = nc.declare_dram_parameter("rel", [P, NT], f32, isOutput=False)
    w1_in = nc.declare_dram_parameter("w1", [DIM, HID], fp8, isOutput=False)
    w2c_in = nc.declare_dram_parameter("w2c", [P, NHH], f32, isOutput=False)
    b1_in = nc.declare_dram_parameter("b1", [HID], f32, isOutput=False)
    iota_in = nc.declare_dram_parameter("iota", [P, P], fp16, isOutput=False)
    ones_in = nc.declare_dram_parameter("ones", [P, 1], fp16, isOutput=False)
    neg25_in = nc.declare_dram_parameter("neg25", [P, 1], f32, isOutput=False)
    u_out = nc.declare_dram_parameter("u", [NW, P, DIM], f32, isOutput=True)
    s_out = nc.declare_dram_parameter("s", [P, NT], f32, isOutput=True)

    win_start = {a: w for w, (a, b) in enumerate(windows)}
    win_end = {b - 1: w for w, (a, b) in enumerate(windows)}
    tile_win = {}
    for w, (a, b) in enumerate(windows):
        for t in range(a, b):
            tile_win[t] = w

    with tile.TileContext(nc) as tc:
        with tc.tile_pool(name="const", bufs=1) as const, \
             tc.tile_pool(name="xpool", bufs=8) as xpool, \
             tc.tile_pool(name="xtpool", bufs=6) as xtpool, \
             tc.tile_pool(name="thpool", bufs=4) as thpool, \
             tc.tile_pool(name="mpool", bufs=4) as mpool, \
             tc.tile_pool(name="apool", bufs=4) as apool, \
             tc.tile_pool(name="mgpool", bufs=4) as mgpool, \
             tc.tile_pool(name="spool", bufs=3) as spool, \
             tc.tile_pool(name="opool", bufs=2) as opool, \
             tc.tile_pool(name="pp_h", bufs=4, space="PSUM") as pp_h, \
             tc.tile_pool(name="pp_s", bufs=2, space="PSUM") as pp_s, \
             tc.tile_pool(name="pp_u", bufs=1, space="PSUM") as pp_u:

            w1t = const.tile([P, NCH, HID], fp8)
            nc.sync.dma_start(out=w1t[:], in_=w1_in.rearrange("(c p) h -> p c h", p=P))
            w2c = const.tile([P, NHH], f32)
            nc.sync.dma_start(out=w2c[:], in_=w2c_in[:])
            b1t = const.tile([P, NHH], f32)
            nc.sync.dma_start(out=b1t[:], in_=b1_in.rearrange("(c p) -> p c", p=P))
            iota = const.tile([P, P], fp16)
            nc.sync.dma_start(out=iota[:], in_=iota_in[:])
            ones = const.tile([P, 1], fp16)
            nc.sync.dma_start(out=ones[:], in_=ones_in[:])
            neg25 = const.tile([P, 1], f32)
            nc.sync.dma_start(out=neg25[:], in_=neg25_in[:])
            relt = const.tile([P, NT], f32)
            nc.sync.dma_start(out=relt[:], in_=rel_in[:])
            s_stage = const.tile([P, NT], f32)

            state = {}
            ugroups = {}

            def emit_mlp(g):
                xg = xpool.tile([P, GRP, DIM], fp16, tag="xg")
                nc.sync.dma_start(out=xg[:].rearrange("p t d -> p (t d)"), in_=x_in[g])
                xTg = xtpool.tile([P, NCH, GRP * P], fp8, tag="xTg")
                nc.sync.dma_start(out=xTg[:].rearrange("p c n -> p (c n)"), in_=xT_in[g])

                thT = thpool.tile([P, NHH, GRP * P], fp16, tag="thT")
                for hh in range(NHH):
                    hTp = pp_h.tile([P, GRP * P], f32, tag="hT", name=f"hTp{hh}")
                    for c in range(0, NCH, 2):
                        nc.tensor.matmul(
                            hTp[:],
                            lhsT=w1t[:, c:c + 2, hh * P:(hh + 1) * P],
                            rhs=xTg[:, c:c + 2],
                            start=(c == 0), stop=(c == NCH - 2),
                            perf_mode=mybir.MatmulPerfMode.DoubleRow)
                    nc.scalar.activation(
                        thT[:, hh], hTp[:],
                        mybir.ActivationFunctionType.Tanh,
                        bias=b1t[:, hh:hh + 1])
                state[g] = {"xg": xg, "thT": thT}

            def emit_score1(g):
                t0 = g * GRP
                st = state[g]
                ctx = tc.high_priority()
                ctx.__enter__()
                # one-hot masks (constants only -> never blocks the DVE queue)
                mg = mgpool.tile([P, GRP, P], fp16, tag="mg")
                for i in range(GRP):
                    nc.vector.tensor_scalar(
                        out=mg[:, i],
                        in0=iota[:],
                        scalar1=relt[:, t0 + i:t0 + i + 1],
                        scalar2=float(b2_plus_25),
                        op0=mybir.AluOpType.is_equal,
                        op1=mybir.AluOpType.mult)
                # mm[p, n] = w2[p]*th0[p, n] + w2[128+p]*th1[p, n]
                m0 = mpool.tile([P, GRP * P], fp16, tag="m0")
                nc.vector.tensor_scalar(
                    out=m0[:], in0=st["thT"][:, 0], scalar1=w2c[:, 0:1],
                    scalar2=None, op0=mybir.AluOpType.mult)
                mm = mpool.tile([P, GRP * P], fp16, tag="mm")
                nc.vector.scalar_tensor_tensor(
                    out=mm[:], in0=st["thT"][:, 1], scalar=w2c[:, 1:2],
                    in1=m0[:], op0=mybir.AluOpType.mult,
                    op1=mybir.AluOpType.add)
                st["mg"] = mg
                st["mm"] = mm
                ctx.__exit__(None, None, None)

            def emit_colsum(g):
                st = state[g]
                # s per node, transposed onto partitions via colsum matmul
                sp = pp_s.tile([P, GRP], f32, tag="sp")
                for i in range(GRP):
                    nc.tensor.matmul(
                        sp[:, i:i + 1],
                        lhsT=st["mm"][:, i * P:(i + 1) * P],
                        rhs=ones[:],
                        start=True, stop=True)
                st["sp"] = sp

            def emit_score2(g):
                t0 = g * GRP
                st = state[g]
                ctx = tc.high_priority()
                ctx.__enter__()
                # s-25 lands in the persistent staging tile (shipped to host,
                # which rebuilds e = exp(s) and the denominators from it) and
                # doubles as the per-tile exp bias. On ScalarE: keeps the DVE
                # queue free of ops that wait on same-iteration PE results.
                nc.scalar.activation(
                    s_stage[:, t0:t0 + GRP], st["sp"][:],
                    mybir.ActivationFunctionType.Identity,
                    bias=neg25[:, 0:1])
                A = apool.tile([P, GRP, P], fp16, tag="A")
                for i in range(GRP):
                    nc.scalar.activation(
                        A[:, i], st["mg"][:, i],
                        mybir.ActivationFunctionType.Exp,
                        bias=s_stage[:, t0 + i:t0 + i + 1])
                st["A"] = A
                ctx.__exit__(None, None, None)

            def emit_scatter(g):
                ts = [g * GRP + i for i in range(GRP)]
                st = state[g]
                for i, t in enumerate(ts):
                    xt = st["xg"][:, i]
                    w = tile_win[t]
                    if t in win_start:
                        uwin = pp_u.tile([P, DIM], f32, tag="uwin")
                        ugroups[w] = uwin
                    up = ugroups[w]
                    for half in range(2):
                        nc.tensor.matmul(
                            up[:, half * 512:(half + 1) * 512],
                            lhsT=st["A"][:, i],
                            rhs=xt[:, half * 512:(half + 1) * 512],
                            start=(t in win_start), stop=(t in win_end))
                    if t in win_end:
                        us = opool.tile([P, DIM], f32, tag="us")
                        nc.vector.tensor_copy(us[:, 0:512], up[:, 0:512])
                        nc.scalar.copy(us[:, 512:1024], up[:, 512:1024])
                        nc.sync.dma_start(out=u_out[w], in_=us[:])
                del state[g]

            # per iteration g: every PE input is >=1 full iteration old.
            #   Scalar: sb(g-3)+exp(g-3) first, then tanh(g)
            #   PE:  mlp(g), colsum(g-2), scatter(g-5)
            #   DVE: mg/m0/mm(g-1), e-reduce(g-5)
            for g in range(NG + 5):
                if 3 <= g < NG + 3:
                    emit_score2(g - 3)
                if g < NG:
                    emit_mlp(g)
                if 1 <= g < NG + 1:
                    emit_score1(g - 1)
                if 2 <= g < NG + 2:
                    emit_colsum(g - 2)
                if 5 <= g:
                    emit_scatter(g - 5)

            nc.sync.dma_start(out=s_out[:], in_=s_stage[:])

    return nc


# ---------------------------------------------------------------------------
# host wrapper
# ---------------------------------------------------------------------------

def _make_windows(NT, nw):
    base, rem = divmod(NT, nw)
    sizes = [base + (1 if i < rem else 0) for i in range(nw)]
    out, a = [], 0
    for s in sizes:
        out.append((a, a + s))
        a += s
    return out


def _reference_numpy(x, batch, W1, b1, W2, b2):
    """Fallback for inputs outside this kernel's structural assumptions."""
    h = np.tanh(x.astype(np.float64) @ W1.astype(np.float64) + b1)
    s = (h @ W2.astype(np.float64) + b2).ravel()
    e = np.exp(s - s.max())
    denom = np.zeros(NUM_SEG, dtype=np.float64)
    np.add.at(denom, batch, e)
    attn = e / (denom[batch] + 1e-8)
    out = np.zeros((NUM_SEG, x.shape[1]), dtype=np.float64)
    np.add.at(out, batch, attn[:, None] * x.astype(np.float64))
    return out.astype(np.float32)


def kernel(x, batch, W1, b1, W2, b2):
    x = np.ascontiguousarray(np.asarray(x, dtype=np.float32))
    batch64 = np.asarray(batch).astype(np.int64)
    W1 = np.asarray(W1, dtype=np.float32)
    b1 = np.asarray(b1, dtype=np.float32)
    W2 = np.asarray(W2, dtype=np.float32)
    b2 = np.asarray(b2, dtype=np.float32)

    N = x.shape[0]
    pc = N // N_CORES
    NT = pc // P
    NG = NT // GRP if NT else 0

    ok = (N == N_CORES * pc and pc == NT * P and NT % GRP == 0
          and x.shape[1] == DIM and W1.shape == (DIM, HID)
          and np.all(batch64[:-1] <= batch64[1:])
          and batch64.min() >= 0 and batch64.max() < NUM_SEG)
    if not ok:
        return _reference_numpy(x, batch64, W1, b1, W2, b2)

    windows = _make_windows(NT, 3)
    iota = np.tile(np.arange(P, dtype=np.float16), (P, 1))
    b2f = float(b2.reshape(-1)[0])
    w2col = np.ascontiguousarray(
        W2.reshape(NHH, P).T.astype(np.float32))          # [P, NHH]
    ones16 = np.ones((P, 1), dtype=np.float16)

    in_maps, meta = [], []
    for c in range(N_CORES):
        bb = batch64[c * pc:(c + 1) * pc]
        rel = np.empty((NT, P), dtype=np.float32)
        bases = []
        for w, (a, b) in enumerate(windows):
            base = int(bb[a * P])
            bases.append(base)
            seg_rel = bb[a * P:b * P] - base
            if seg_rel.min() < 0 or seg_rel.max() >= P:
                return _reference_numpy(x, batch64, W1, b1, W2, b2)
            rel[a:b] = seg_rel.reshape(b - a, P).astype(np.float32)
        xc = x[c * pc:(c + 1) * pc]
        x16 = xc.astype(np.float16)
        x8 = xc.astype(ml_dtypes.float8_e4m3)
        xt = np.ascontiguousarray(
            x16.reshape(NG, GRP, P, DIM).transpose(0, 2, 1, 3)
               .reshape(NG, P, GRP * DIM))
        xTt = np.ascontiguousarray(
            x8.reshape(NG, GRP * P, NCH, P).transpose(0, 3, 2, 1)
               .reshape(NG, P, NCH * GRP * P))
        in_maps.append({
            "x": xt,
            "xT": xTt,
            "rel": np.ascontiguousarray(rel.T),
            "w1": W1.astype(ml_dtypes.float8_e4m3),
            "w2c": w2col,
            "b1": b1,
            "iota": iota,
            "ones": ones16,
            "neg25": np.full((P, 1), -25.0, dtype=np.float32),
        })
        meta.append(bases)

    _install_ntff_hook()
    _install_tile_compat()
    from concourse.bass_utils import run_bass_kernel_spmd

    nc = _build_kernel(NT, windows, b2f + 25.0)
    _split_multi_waits(nc)

    trace = os.environ.get("KERNEL_TRACE", "") == "1"
    res = run_bass_kernel_spmd(nc, in_maps, list(range(N_CORES)), trace=trace)
    if trace and res.exec_time_ns:
        print(f"[kernel] HW exec time: {res.exec_time_ns} ns", file=sys.stderr)
        kernel.last_exec_time_ns = res.exec_time_ns

    # host unshard: accumulate windows, build denominators from e, divide
    u_sum = np.zeros((NUM_SEG, DIM), dtype=np.float64)
    e_full = np.empty(N, dtype=np.float64)
    for c in range(N_CORES):
        r = res.results[c]
        s25 = r["s"].T.reshape(-1).astype(np.float64)
        e_full[c * pc:(c + 1) * pc] = np.exp(
            s25 + 25.0 + np.float64(np.float16(b2f + 25.0)) - 25.0)
        for w in range(len(windows)):
            base = meta[c][w]
            hi = min(base + P, NUM_SEG)
            u_sum[base:hi] += r["u"][w][:hi - base]
    denom = np.zeros(NUM_SEG, dtype=np.float64)
    np.add.at(denom, batch64, e_full)
    s_max = float(np.log(max(e_full.max(), 1e-30)))
    out = u_sum / (denom + 1e-8 * np.exp(s_max))[:, None]
    return out.astype(np.float32)


kernel.last_exec_time_ns = None
